# revision 20
# baseline (speedup 1.0000x reference)
"""Causal self-attention (B=1, T=4096, C=1024, H=8) on 8 trn2 NeuronCores.

Tensor-parallel over heads: core h owns head h (D=128 = partition width).
Feature-major layout throughout: PE contraction dim always on SBUF
partitions.

Structure: 8 chunks of TQ=512 queries, software-pipelined attention
loop over s-tile PAIRS with fp8 DoubleRow matmuls where the
contraction dim allows pairing (2x PE columns/cycle):

  chunk c (queries t0=512c .. t0+511, s-tile pairs g = 0..2c+1):
    S(2g), S(2g+1) = kT-block.T @ qT        [PE bf16, 512 cols each]
    exp over the pair [128,1024] on ACT      (scale=1/(SQ*SK) folds the
                                              fp8 weight scaling out)
    clean pairs: p2 in fp8 -> U/A as DoubleRow fp8 matmuls (2 s-tiles
                 per instruction)
    diag pairs (last 2): p2 in bf16, DVE mask-mul, plain bf16 U/A
  emission per pair:  S(g+1) | filler MMs | exp(g+1) | U(g) A(g)

  QKV uses fp8 DoubleRow too (x and w_qkv in fp8, scaled by SQ/SK/SV
  to dodge fp8 subnormals; 1/(SQ*SK) folded into the exp scale,
  1/SV folded into w_proj on the host).

  Filler = QKV(c+1) matmuls + v(c+1) transposes + c_proj(c-1) matmuls,
  paced evenly across the pair loop so the PE never drains (keeps HAM
  at full clock).  DMA: inputs on the sync HWDGE ring; outputs (bf16
  partials, host sums in f32) on the gpsimd SWDGE ring.
"""

import math
import os
import sys

for _p in ("/opt/trn_rl_repo",):
    if _p not in sys.path:
        sys.path.insert(0, _p)

import numpy as np
import ml_dtypes

import concourse.bass as bass
import concourse.mybir as mybir
import concourse.tile as tile
from concourse import bacc
from concourse import bass_utils
from concourse.masks import make_identity

B, T, C, H = 1, 4096, 1024, 8
D = C // H          # 128, head dim == partition width
N_CORES = 8
TQ = 512            # query-chunk
NCH = T // TQ       # 8 chunks
CO = C // 128       # 8 contraction tiles of 128
F32 = mybir.dt.float32
BF16 = mybir.dt.bfloat16
FP8 = mybir.dt.float8e4
DR = mybir.MatmulPerfMode.DoubleRow

P_DT = BF16         # qT/kT storage
OUT_DT = BF16       # outP partial payload (host sums in f32)

# fp8 scaling: keep weight/act values out of e4m3 subnormals (<2^-6)
# and below the TRN e4m3 max of 240.
SQ = 64.0           # wq (incl 1/sqrt(D)) and bq
SK = 64.0           # wk, bk
SV = 32.0           # wv, bv; 1/SV folded into w_proj host-side
EXP_SCALE = 1.0 / (SQ * SK)


def build(t_len=T):
    """Emit the single-core SPMD program (same code on all 8 cores)."""
    n_ttiles = t_len // 128          # 32 s-tiles
    nch = t_len // TQ
    nc = bacc.Bacc(
        "TRN2", target_bir_lowering=False, debug=False, num_devices=N_CORES
    )

    xT_d = nc.dram_tensor("xT", [C, t_len], FP8, kind="ExternalInput")
    # chunk 0 runs QKV in bf16: its queries have few-term softmax
    # denominators, so fp8 projection error passes straight through
    xT16_d = nc.dram_tensor("xT16", [C, TQ], BF16, kind="ExternalInput")
    wq_d = nc.dram_tensor("wq", [C, D], FP8, kind="ExternalInput")
    wk_d = nc.dram_tensor("wk", [C, D], FP8, kind="ExternalInput")
    wv_d = nc.dram_tensor("wv", [C, D], FP8, kind="ExternalInput")
    wq16_d = nc.dram_tensor("wq16", [C, D], BF16, kind="ExternalInput")
    wk16_d = nc.dram_tensor("wk16", [C, D], BF16, kind="ExternalInput")
    wv16_d = nc.dram_tensor("wv16", [C, D], BF16, kind="ExternalInput")
    wp_d = nc.dram_tensor("wp", [D, C], BF16, kind="ExternalInput")
    bqkv_d = nc.dram_tensor("bqkv", [D, 3], F32, kind="ExternalInput")
    outP_d = nc.dram_tensor("outP", [C, t_len], OUT_DT, kind="ExternalOutput")

    with tile.TileContext(nc) as tc:
        with (
            tc.tile_pool(name="const", bufs=1) as cpool,
            tc.tile_pool(name="persist", bufs=1) as ppool,
            tc.tile_pool(name="work", bufs=2) as wpool,
            tc.tile_pool(name="psum", bufs=1, space="PSUM") as psum,
        ):
            # ---- weights / constants -------------------------------------
            wq_sb = cpool.tile([128, CO, D], FP8, name="wq_sb")
            wk_sb = cpool.tile([128, CO, D], FP8, name="wk_sb")
            wv_sb = cpool.tile([128, CO, D], FP8, name="wv_sb")
            wq16_sb = cpool.tile([128, CO, D], BF16, name="wq16_sb")
            wk16_sb = cpool.tile([128, CO, D], BF16, name="wk16_sb")
            wv16_sb = cpool.tile([128, CO, D], BF16, name="wv16_sb")
            wp_sb = cpool.tile([128, CO, D], BF16, name="wp_sb")
            bqkv_sb = cpool.tile([D, 3], F32, name="bqkv_sb")
            xc0_16 = cpool.tile([128, CO, TQ], BF16, name="xc0_16")
            # prologue inputs split across the two HWDGE rings (sync +
            # scalar) so the serialized per-DMA cost halves at the ramp
            nc.sync.dma_start(
                wq16_sb[:], wq16_d.ap().rearrange("(o p) m -> p o m", p=128)
            )
            nc.scalar.dma_start(bqkv_sb[:], bqkv_d.ap())
            nc.sync.dma_start(
                xc0_16[:, 0:4, :],
                xT16_d.ap().rearrange("(o p) t -> p o t", p=128)[:, 0:4, :])
            nc.sync.dma_start(
                xc0_16[:, 4:8, :],
                xT16_d.ap().rearrange("(o p) t -> p o t", p=128)[:, 4:8, :])
            for w_sb, w_d in ((wk16_sb, wk16_d), (wv16_sb, wv16_d)):
                nc.scalar.dma_start(
                    w_sb[:], w_d.ap().rearrange("(o p) m -> p o m", p=128)
                )

            xT_blk = xT_d.ap().rearrange("(o p) t -> p o t", p=128)
            outP_blk = outP_d.ap().rearrange("(o p) t -> p o t", p=128)

            # x chunk ring: [128, CO, TQ] per chunk, 3 deep
            def xc_fetch(c):
                xc = wpool.tile([128, CO, TQ], FP8, tag="xc", name="xc", bufs=4)
                t0 = c * TQ
                nc.sync.dma_start(xc[:, 0:4, :], xT_blk[:, 0:4, t0 : t0 + TQ])
                nc.sync.dma_start(xc[:, 4:8, :], xT_blk[:, 4:8, t0 : t0 + TQ])
                return xc

            for w_sb, w_d in ((wq_sb, wq_d), (wk_sb, wk_d), (wv_sb, wv_d)):
                nc.scalar.dma_start(
                    w_sb[:], w_d.ap().rearrange("(o p) m -> p o m", p=128)
                )
            xcs = {1: xc_fetch(1)}
            nc.scalar.dma_start(
                wp_sb[:], wp_d.ap().rearrange("d (o j) -> d o j", j=128)
            )

            masks = cpool.tile([128, 4, TQ], P_DT, name="masks")
            nc.vector.memset(masks[:], 1.0)
            for j in range(4):
                nc.gpsimd.affine_select(
                    out=masks[:, j, :], in_=masks[:, j, :],
                    compare_op=mybir.AluOpType.is_ge, fill=0.0,
                    base=-128 * j, pattern=[[1, TQ]], channel_multiplier=-1,
                )
            ones_sq = cpool.tile([128, 128], P_DT, name="ones_sq")
            nc.vector.memset(ones_sq[:], 1.0)
            ones_dr = cpool.tile([128, 2, 128], FP8, name="ones_dr")
            nc.vector.memset(ones_dr[:], 1.0)
            ident = cpool.tile([128, 128], P_DT, name="ident")
            make_identity(nc, ident[:])

            # HAM/ifetch warmup: dummy matmuls while input DMAs land
            warm_ps = psum.tile([128, 128], F32, tag="oh", name="warm_ps",
                                bufs=1)
            for wi in range(32):
                nc.tensor.matmul(warm_ps[:], ones_sq[:], ones_sq[:],
                                 start=True, stop=True)

            # ---- persistent activations ----------------------------------
            kT_sb = ppool.tile([128, t_len], P_DT, name="kT_sb")
            v_sb = ppool.tile([128, n_ttiles, D], FP8, name="v_sb")

            yT_ring = {}     # chunk -> yT tile [128, TQ]
            qT_ring = {}     # chunk -> qT tile [128, TQ]
            v16_ring = {}    # chunk -> bf16 v tiles [128, 4, 128] (diag)

            # ---------------- emission helpers ----------------------------
            def qkv_thunks(c):
                """Filler thunks computing q/k/v for chunk c from xc.
                Chunk 0 uses the bf16 path (precision: its queries have
                few-term denominators); later chunks use fp8 DoubleRow."""
                bf = c == 0
                xc = xc0_16 if bf else xcs[c]
                t0 = c * TQ
                thunks = []

                def proj(w_sb, kind):
                    ps = psum.tile([128, TQ], F32, tag="qkv",
                                   name=f"{kind}ps", bufs=1)
                    if bf:
                        for o in range(CO):
                            def mm(o=o, ps=ps, w_sb=w_sb):
                                nc.tensor.matmul(
                                    ps[:], w_sb[:, o, :], xc[:, o, :],
                                    start=(o == 0), stop=(o == CO - 1),
                                )
                            thunks.append(mm)
                    else:
                        for o2 in range(4):
                            def mm(o2=o2, ps=ps, w_sb=w_sb):
                                nc.tensor.matmul(
                                    ps[:], w_sb[:, 2 * o2 : 2 * o2 + 2, :],
                                    xc[:, 2 * o2 : 2 * o2 + 2, :],
                                    start=(o2 == 0), stop=(o2 == 3),
                                    perf_mode=DR,
                                )
                            thunks.append(mm)

                    def finish(ps=ps, kind=kind):
                        if kind == "q":
                            qT = wpool.tile([128, TQ], P_DT, tag="qT",
                                            name="qT", bufs=2)
                            nc.vector.tensor_add(
                                qT[:], ps[:],
                                bqkv_sb[:, 0:1].to_broadcast([D, TQ]))
                            qT_ring[c] = qT
                        elif kind == "k":
                            nc.vector.tensor_add(
                                kT_sb[:, t0 : t0 + TQ], ps[:],
                                bqkv_sb[:, 1:2].to_broadcast([D, TQ]))
                        else:
                            vT = wpool.tile([128, TQ], P_DT, tag="vT",
                                            name="vT", bufs=2)
                            nc.vector.tensor_add(
                                vT[:], ps[:],
                                bqkv_sb[:, 2:3].to_broadcast([D, TQ]))
                            # transpose to token-major [s, d] tiles
                            vt_ps = psum.tile([128, 4, 128], P_DT, tag="oh",
                                              name="vt_ps", bufs=1)
                            for tt in range(4):
                                nc.tensor.transpose(
                                    vt_ps[:, tt, :],
                                    vT[:, tt * 128 : (tt + 1) * 128],
                                    ident[:])
                            # fp8 copy for DoubleRow A/U; bf16 copy for
                            # the diagonal (masked) pairs of chunk c
                            nc.vector.tensor_copy(
                                v_sb[:, 4 * c : 4 * c + 4, :], vt_ps[:])
                            v16 = wpool.tile([128, 4, 128], P_DT, tag="v16",
                                             name="v16", bufs=2)
                            nc.vector.tensor_copy(v16[:], vt_ps[:])
                            v16_ring[c] = v16
                    # attach the finish to the last MM thunk
                    last = thunks.pop()
                    def last_plus(last=last, finish=finish):
                        last()
                        finish()
                    thunks.append(last_plus)

                if bf:
                    proj(wq16_sb, "q")
                    proj(wk16_sb, "k")
                    proj(wv16_sb, "v")
                else:
                    proj(wq_sb, "q")
                    proj(wk_sb, "k")
                    proj(wv_sb, "v")
                return thunks

            outc_ring = {}

            def cproj_thunks(c, split_copy=False, epilogue=False):
                """Filler thunks computing the local c_proj partial of
                chunk c (host sums partials over cores).  The epilogue
                variant round-robins psum banks across the now-idle tags
                and quarters the output DMA to shrink the receipt tail."""
                t0 = c * TQ
                yT = yT_ring[c]
                outc = wpool.tile([128, CO, TQ], OUT_DT, tag="outc",
                                  name="outc", bufs=2)
                outc_ring[c] = outc
                tags = (("s2", 2), ("qkv", 1), ("oh", 1), ("yps", 1),
                        ("sps", 1)) if epilogue else (("oh", 1),)
                thunks = []
                for j in range(CO):
                    def mm(j=j):
                        tg, bf = tags[j % len(tags)]
                        oh = psum.tile([128, TQ], F32, tag=tg,
                                       name="oh", bufs=bf)
                        nc.tensor.matmul(
                            oh[:], wp_sb[:, j, :], yT[:],
                            start=True, stop=True)
                        if split_copy and j % 2 == 1:
                            nc.scalar.copy(outc[:, j, :], oh[:])
                        else:
                            nc.vector.tensor_copy(outc[:, j, :], oh[:])
                        if epilogue:
                            if j % 2 == 1:
                                nc.gpsimd.dma_start(
                                    outP_blk[:, j - 1 : j + 1, t0 : t0 + TQ],
                                    outc[:, j - 1 : j + 1, :])
                        elif j == 3:
                            nc.gpsimd.dma_start(
                                outP_blk[:, 0:4, t0 : t0 + TQ],
                                outc[:, 0:4, :])
                        elif j == CO - 1:
                            nc.gpsimd.dma_start(
                                outP_blk[:, 4:8, t0 : t0 + TQ],
                                outc[:, 4:8, :])
                    thunks.append(mm)
                return thunks

            def s2pair_f(qT, g):
                s2p = psum.tile([128, 2, TQ], F32, tag="s2", name="s2p",
                                bufs=2)
                for h in range(2):
                    si = 2 * g + h
                    nc.tensor.matmul(
                        s2p[:, h, :],
                        kT_sb[:, si * 128 : si * 128 + 128], qT[:],
                        start=True, stop=True)
                return s2p

            def exp_pair_f(g, s2p, n_s):
                diag = 2 * g >= n_s - 4
                if diag:
                    p2 = wpool.tile([128, 2, TQ], P_DT, tag="p2b",
                                    name="p2b", bufs=3)
                    nc.scalar.activation(
                        p2[:], s2p[:], mybir.ActivationFunctionType.Exp,
                        scale=EXP_SCALE)
                    for h in range(2):
                        si = 2 * g + h
                        # mask-mul on gpsimd: off the DVE FIFO (which
                        # carries the c_proj casts and bias adds)
                        nc.gpsimd.tensor_mul(
                            p2[:, h, :], p2[:, h, :],
                            masks[:, si - (n_s - 4), :])
                else:
                    p2 = wpool.tile([128, 2, TQ], FP8, tag="p2f",
                                    name="p2f", bufs=4)
                    nc.scalar.activation(
                        p2[:], s2p[:], mybir.ActivationFunctionType.Exp,
                        scale=EXP_SCALE)
                return (p2, diag)

            # ---- prologue: QKV(0) dense ----------------------------------
            for th in qkv_thunks(0):
                th()

            # ---- main chunk loop -----------------------------------------
            hoisted = {}
            for c in range(nch):
                t0 = c * TQ
                n_s = 4 * (c + 1)
                n_p = n_s // 2

                filler = []
                if c + 1 < nch:
                    filler += qkv_thunks(c + 1)
                if c >= 1:
                    filler += cproj_thunks(c - 1, split_copy=(c - 1 <= 3))
                if c == 0:
                    xcs[2] = xc_fetch(2)
                if c + 3 < nch:
                    xcs[c + 3] = xc_fetch(c + 3)

                qT = qT_ring[c]
                v16 = v16_ring[c]
                y_ps = psum.tile([128, TQ], F32, tag="yps", name="y_ps",
                                 bufs=1)
                s_ps = psum.tile([128, TQ], F32, tag="sps", name="s_ps",
                                 bufs=1)

                pps = {}

                def ua_pair(g):
                    p2, diag = pps.pop(g)
                    if diag:
                        for h in range(2):
                            si = 2 * g + h
                            nc.tensor.matmul(
                                s_ps[:], ones_sq[:], p2[:, h, :],
                                start=(si == 0), stop=(si == n_s - 1))
                            nc.tensor.matmul(
                                y_ps[:], v16[:, si - (n_s - 4), :],
                                p2[:, h, :],
                                start=(si == 0), stop=(si == n_s - 1))
                    else:
                        nc.tensor.matmul(
                            s_ps[:], ones_dr[:], p2[:],
                            start=(g == 0), stop=False, perf_mode=DR)
                        nc.tensor.matmul(
                            y_ps[:], v_sb[:, 2 * g : 2 * g + 2, :], p2[:],
                            start=(g == 0), stop=False, perf_mode=DR)

                if c in hoisted:
                    pps[0] = hoisted.pop(c)
                else:
                    pps[0] = exp_pair_f(0, s2pair_f(qT, 0), n_s)
                for g in range(n_p):
                    if g + 1 < n_p:
                        s2_nxt = s2pair_f(qT, g + 1)
                    nf = len(filler)
                    if nf:
                        take = max(1, -(-nf // (n_p - g)))
                        for th in filler[:take]:
                            th()
                        del filler[:take]
                    if g + 1 < n_p:
                        pps[g + 1] = exp_pair_f(g + 1, s2_nxt, n_s)
                    elif c + 1 < nch:
                        # hoist the next chunk's first scores+exp here so
                        # its U/A never waits on a cold exp at the boundary
                        # (qT(c+1) was produced by this chunk's filler;
                        # kT block 0 is ancient)
                        s2h = s2pair_f(qT_ring[c + 1], 0)
                        hoisted[c + 1] = exp_pair_f(0, s2h, 4 * (c + 2))
                    ua_pair(g)

                for th in filler:
                    th()

                recip = wpool.tile([128, TQ], F32, tag="recip", name="recip",
                                   bufs=2)
                nc.vector.reciprocal_approx_fast(recip[:], s_ps[:])
                yT = wpool.tile([128, TQ], P_DT, tag="yT", name="yT", bufs=2)
                nc.vector.tensor_mul(yT[:], y_ps[:], recip[:])
                yT_ring[c] = yT

            # ---- epilogue: last chunk's c_proj, pipelined ----------------
            for th in cproj_thunks(nch - 1, split_copy=True, epilogue=True):
                th()

    nc.compile()
    return nc


def make_in_maps(x, w_attn, b_attn, w_proj, b_proj, t_len=T):
    """Shard + lay out the full inputs for the 8 cores."""
    x = np.asarray(x, dtype=np.float32).reshape(t_len, C)
    w_attn = np.asarray(w_attn, dtype=np.float32)
    b_attn = np.asarray(b_attn, dtype=np.float32)
    w_proj = np.asarray(w_proj, dtype=np.float32)

    scale = 1.0 / math.sqrt(D)
    fp8 = ml_dtypes.float8_e4m3
    bf16 = ml_dtypes.bfloat16
    xT = np.ascontiguousarray(x.T)
    xT8 = xT.astype(fp8)
    xT16 = np.ascontiguousarray(xT[:, :TQ]).astype(bf16)

    in_maps = []
    for h in range(N_CORES):
        sl = slice(h * D, (h + 1) * D)
        wq_s = (w_attn[sl, :] * (scale * SQ)).T
        wk_s = (w_attn[C + h * D : C + (h + 1) * D, :] * SK).T
        wv_s = (w_attn[2 * C + h * D : 2 * C + (h + 1) * D, :] * SV).T
        wp = np.ascontiguousarray((w_proj[:, sl] * (1.0 / SV)).T).astype(bf16)
        bqkv = np.stack(
            [
                b_attn[sl] * (scale * SQ),
                b_attn[C + h * D : C + (h + 1) * D] * SK,
                b_attn[2 * C + h * D : 2 * C + (h + 1) * D] * SV,
            ],
            axis=1,
        ).astype(np.float32)
        in_maps.append({
            "xT": xT8, "xT16": xT16,
            "wq": np.ascontiguousarray(wq_s).astype(fp8),
            "wk": np.ascontiguousarray(wk_s).astype(fp8),
            "wv": np.ascontiguousarray(wv_s).astype(fp8),
            "wq16": np.ascontiguousarray(wq_s).astype(bf16),
            "wk16": np.ascontiguousarray(wk_s).astype(bf16),
            "wv16": np.ascontiguousarray(wv_s).astype(bf16),
            "wp": wp,
            "bqkv": np.ascontiguousarray(bqkv),
        })
    return in_maps


_COMPILED = {}


def _get_compiled(t_len=T):
    if t_len not in _COMPILED:
        _COMPILED[t_len] = build(t_len)
    return _COMPILED[t_len]


def kernel(x, w_attn, b_attn, w_proj, b_proj, trace=False):
    nc = _get_compiled()
    in_maps = make_in_maps(x, w_attn, b_attn, w_proj, b_proj)
    res = bass_utils.run_bass_kernel_spmd(
        nc, in_maps, core_ids=list(range(N_CORES)), trace=trace
    )
    acc = res.results[0]["outP"].astype(np.float32)
    for h in range(1, N_CORES):
        acc += res.results[h]["outP"].astype(np.float32)
    out = acc.T + np.asarray(b_proj, dtype=np.float32)
    out = np.ascontiguousarray(out, dtype=np.float32).reshape(B, T, C)
    if trace:
        kernel.last_exec_time_ns = res.exec_time_ns
        kernel.last_results = res
    return out


# revision 21
# speedup vs baseline: 1.0609x; 1.0609x over previous
"""Causal self-attention (B=1, T=4096, C=1024, H=8) on 8 trn2 NeuronCores.

Tensor-parallel over heads: core h owns head h (D=128 = partition width).
Feature-major layout throughout: PE contraction dim always on SBUF
partitions.

Structure: 8 chunks of TQ=512 queries, software-pipelined attention
loop over s-tile PAIRS with fp8 DoubleRow matmuls where the
contraction dim allows pairing (2x PE columns/cycle):

  chunk c (queries t0=512c .. t0+511, s-tile pairs g = 0..2c+1):
    S(2g), S(2g+1) = kT-block.T @ qT        [PE bf16, 512 cols each]
    exp over the pair [128,1024] on ACT      (scale=1/(SQ*SK) folds the
                                              fp8 weight scaling out)
    clean pairs: p2 in fp8 -> U/A as DoubleRow fp8 matmuls (2 s-tiles
                 per instruction)
    diag pairs (last 2): p2 in bf16, DVE mask-mul, plain bf16 U/A
  emission per pair:  S(g+1) | filler MMs | exp(g+1) | U(g) A(g)

  QKV uses fp8 DoubleRow too (x and w_qkv in fp8, scaled by SQ/SK/SV
  to dodge fp8 subnormals; 1/(SQ*SK) folded into the exp scale,
  1/SV folded into w_proj on the host).

  Filler = QKV(c+1) matmuls + v(c+1) transposes + c_proj(c-1) matmuls,
  paced evenly across the pair loop so the PE never drains (keeps HAM
  at full clock).  DMA: inputs on the sync HWDGE ring; outputs (bf16
  partials, host sums in f32) on the gpsimd SWDGE ring.
"""

import math
import os
import sys

for _p in ("/opt/trn_rl_repo",):
    if _p not in sys.path:
        sys.path.insert(0, _p)

import numpy as np
import ml_dtypes

import concourse.bass as bass
import concourse.mybir as mybir
import concourse.tile as tile
from concourse import bacc
from concourse import bass_utils
from concourse.masks import make_identity

B, T, C, H = 1, 4096, 1024, 8
D = C // H          # 128, head dim == partition width
N_CORES = 8
TQ = 512            # query-chunk
NCH = T // TQ       # 8 chunks
CO = C // 128       # 8 contraction tiles of 128
F32 = mybir.dt.float32
BF16 = mybir.dt.bfloat16
FP8 = mybir.dt.float8e4
DR = mybir.MatmulPerfMode.DoubleRow

P_DT = BF16         # qT/kT storage
OUT_DT = BF16       # outP partial payload (host sums in f32)

# fp8 scaling: keep weight/act values out of e4m3 subnormals (<2^-6)
# and below the TRN e4m3 max of 240.
SQ = 64.0           # wq (incl 1/sqrt(D)) and bq
SK = 64.0           # wk, bk
SV = 32.0           # wv, bv; 1/SV folded into w_proj host-side
EXP_SCALE = 1.0 / (SQ * SK)


def build(t_len=T):
    """Emit the single-core SPMD program (same code on all 8 cores)."""
    n_ttiles = t_len // 128          # 32 s-tiles
    nch = t_len // TQ
    nc = bacc.Bacc(
        "TRN2", target_bir_lowering=False, debug=False, num_devices=N_CORES
    )

    xT_d = nc.dram_tensor("xT", [C, t_len], FP8, kind="ExternalInput")
    # chunk 0 runs QKV in bf16: its queries have few-term softmax
    # denominators, so fp8 projection error passes straight through
    xT16_d = nc.dram_tensor("xT16", [C, TQ], BF16, kind="ExternalInput")
    wq_d = nc.dram_tensor("wq", [C, D], FP8, kind="ExternalInput")
    wk_d = nc.dram_tensor("wk", [C, D], FP8, kind="ExternalInput")
    wv_d = nc.dram_tensor("wv", [C, D], FP8, kind="ExternalInput")
    wq16_d = nc.dram_tensor("wq16", [C, D], BF16, kind="ExternalInput")
    wk16_d = nc.dram_tensor("wk16", [C, D], BF16, kind="ExternalInput")
    wv16_d = nc.dram_tensor("wv16", [C, D], BF16, kind="ExternalInput")
    wp_d = nc.dram_tensor("wp", [D, C], BF16, kind="ExternalInput")
    bqkv_d = nc.dram_tensor("bqkv", [D, 3], F32, kind="ExternalInput")
    outP_d = nc.dram_tensor("outP", [C, t_len], OUT_DT, kind="ExternalOutput")

    with tile.TileContext(nc) as tc:
        with (
            tc.tile_pool(name="const", bufs=1) as cpool,
            tc.tile_pool(name="persist", bufs=1) as ppool,
            tc.tile_pool(name="work", bufs=2) as wpool,
            tc.tile_pool(name="psum", bufs=1, space="PSUM") as psum,
        ):
            # ---- weights / constants -------------------------------------
            wq_sb = cpool.tile([128, CO, D], FP8, name="wq_sb")
            wk_sb = cpool.tile([128, CO, D], FP8, name="wk_sb")
            wv_sb = cpool.tile([128, CO, D], FP8, name="wv_sb")
            wq16_sb = cpool.tile([128, CO, D], BF16, name="wq16_sb")
            wk16_sb = cpool.tile([128, CO, D], BF16, name="wk16_sb")
            wv16_sb = cpool.tile([128, CO, D], BF16, name="wv16_sb")
            wp_sb = cpool.tile([128, CO, D], BF16, name="wp_sb")
            bqkv_sb = cpool.tile([D, 3], F32, name="bqkv_sb")
            xc0_16 = cpool.tile([128, CO, TQ], BF16, name="xc0_16")
            # prologue inputs split across the two HWDGE rings (sync +
            # scalar) so the serialized per-DMA cost halves at the ramp
            nc.sync.dma_start(
                wq16_sb[:], wq16_d.ap().rearrange("(o p) m -> p o m", p=128)
            )
            nc.scalar.dma_start(bqkv_sb[:], bqkv_d.ap())
            nc.sync.dma_start(
                xc0_16[:, 0:4, :],
                xT16_d.ap().rearrange("(o p) t -> p o t", p=128)[:, 0:4, :])
            nc.sync.dma_start(
                xc0_16[:, 4:8, :],
                xT16_d.ap().rearrange("(o p) t -> p o t", p=128)[:, 4:8, :])
            for w_sb, w_d in ((wk16_sb, wk16_d), (wv16_sb, wv16_d)):
                nc.scalar.dma_start(
                    w_sb[:], w_d.ap().rearrange("(o p) m -> p o m", p=128)
                )

            xT_blk = xT_d.ap().rearrange("(o p) t -> p o t", p=128)
            outP_blk = outP_d.ap().rearrange("(o p) t -> p o t", p=128)

            # x chunk ring: [128, CO, TQ] per chunk, 3 deep
            def xc_fetch(c):
                xc = wpool.tile([128, CO, TQ], FP8, tag="xc", name="xc", bufs=4)
                t0 = c * TQ
                nc.sync.dma_start(xc[:, 0:4, :], xT_blk[:, 0:4, t0 : t0 + TQ])
                nc.sync.dma_start(xc[:, 4:8, :], xT_blk[:, 4:8, t0 : t0 + TQ])
                return xc

            for w_sb, w_d in ((wq_sb, wq_d), (wk_sb, wk_d), (wv_sb, wv_d)):
                nc.scalar.dma_start(
                    w_sb[:], w_d.ap().rearrange("(o p) m -> p o m", p=128)
                )
            xcs = {1: xc_fetch(1)}
            nc.scalar.dma_start(
                wp_sb[:], wp_d.ap().rearrange("d (o j) -> d o j", j=128)
            )

            masks = cpool.tile([128, 4, TQ], P_DT, name="masks")
            nc.vector.memset(masks[:], 1.0)
            for j in range(4):
                nc.gpsimd.affine_select(
                    out=masks[:, j, :], in_=masks[:, j, :],
                    compare_op=mybir.AluOpType.is_ge, fill=0.0,
                    base=-128 * j, pattern=[[1, TQ]], channel_multiplier=-1,
                )
            ones_sq = cpool.tile([128, 128], P_DT, name="ones_sq")
            nc.vector.memset(ones_sq[:], 1.0)
            ones_dr = cpool.tile([128, 2, 128], FP8, name="ones_dr")
            nc.vector.memset(ones_dr[:], 1.0)
            ident = cpool.tile([128, 128], P_DT, name="ident")
            make_identity(nc, ident[:])

            # HAM/ifetch warmup: dummy matmuls while input DMAs land
            warm_ps = psum.tile([128, 128], F32, tag="oh", name="warm_ps",
                                bufs=1)
            for wi in range(32):
                nc.tensor.matmul(warm_ps[:], ones_sq[:], ones_sq[:],
                                 start=True, stop=True)

            # ---- persistent activations ----------------------------------
            kT_sb = ppool.tile([128, t_len], P_DT, name="kT_sb")
            v_sb = ppool.tile([128, n_ttiles, D], FP8, name="v_sb")

            yT_ring = {}     # chunk -> yT tile [128, TQ]
            qT_ring = {}     # chunk -> qT tile [128, TQ]
            v16_ring = {}    # chunk -> bf16 v tiles [128, 4, 128] (diag)

            # ---------------- emission helpers ----------------------------
            def qkv_thunks(c):
                """Filler thunks computing q/k/v for chunk c from xc.
                Chunk 0 uses the bf16 path (precision: its queries have
                few-term denominators); later chunks use fp8 DoubleRow."""
                bf = c == 0
                xc = xc0_16 if bf else xcs[c]
                t0 = c * TQ
                thunks = []

                def proj(w_sb, kind):
                    ps = psum.tile([128, TQ], F32, tag="qkv",
                                   name=f"{kind}ps", bufs=1)
                    if bf:
                        for o in range(CO):
                            def mm(o=o, ps=ps, w_sb=w_sb):
                                nc.tensor.matmul(
                                    ps[:], w_sb[:, o, :], xc[:, o, :],
                                    start=(o == 0), stop=(o == CO - 1),
                                )
                            thunks.append(mm)
                    else:
                        for o2 in range(4):
                            def mm(o2=o2, ps=ps, w_sb=w_sb):
                                nc.tensor.matmul(
                                    ps[:], w_sb[:, 2 * o2 : 2 * o2 + 2, :],
                                    xc[:, 2 * o2 : 2 * o2 + 2, :],
                                    start=(o2 == 0), stop=(o2 == 3),
                                    perf_mode=DR,
                                )
                            thunks.append(mm)

                    def finish(ps=ps, kind=kind):
                        if kind == "q":
                            qT = wpool.tile([128, TQ], P_DT, tag="qT",
                                            name="qT", bufs=2)
                            nc.vector.tensor_add(
                                qT[:], ps[:],
                                bqkv_sb[:, 0:1].to_broadcast([D, TQ]))
                            qT_ring[c] = qT
                        elif kind == "k":
                            nc.vector.tensor_add(
                                kT_sb[:, t0 : t0 + TQ], ps[:],
                                bqkv_sb[:, 1:2].to_broadcast([D, TQ]))
                        else:
                            vT = wpool.tile([128, TQ], P_DT, tag="vT",
                                            name="vT", bufs=2)
                            nc.vector.tensor_add(
                                vT[:], ps[:],
                                bqkv_sb[:, 2:3].to_broadcast([D, TQ]))
                            # transpose to token-major [s, d] tiles
                            vt_ps = psum.tile([128, 4, 128], P_DT, tag="oh",
                                              name="vt_ps", bufs=1)
                            for tt in range(4):
                                nc.tensor.transpose(
                                    vt_ps[:, tt, :],
                                    vT[:, tt * 128 : (tt + 1) * 128],
                                    ident[:])
                            # fp8 copy for DoubleRow A/U; bf16 copy for
                            # the diagonal (masked) pairs of chunk c
                            nc.vector.tensor_copy(
                                v_sb[:, 4 * c : 4 * c + 4, :], vt_ps[:])
                            v16 = wpool.tile([128, 4, 128], P_DT, tag="v16",
                                             name="v16", bufs=2)
                            nc.vector.tensor_copy(v16[:], vt_ps[:])
                            v16_ring[c] = v16
                    # attach the finish to the last MM thunk
                    last = thunks.pop()
                    def last_plus(last=last, finish=finish):
                        last()
                        finish()
                    thunks.append(last_plus)

                if bf:
                    proj(wq16_sb, "q")
                    proj(wk16_sb, "k")
                    proj(wv16_sb, "v")
                else:
                    proj(wq_sb, "q")
                    proj(wk_sb, "k")
                    proj(wv_sb, "v")
                return thunks

            outc_ring = {}

            def cproj_thunks(c, split_copy=False, epilogue=False):
                """Filler thunks computing the local c_proj partial of
                chunk c (host sums partials over cores).  The epilogue
                variant round-robins psum banks across the now-idle tags
                and quarters the output DMA to shrink the receipt tail."""
                t0 = c * TQ
                yT = yT_ring[c]
                outc = wpool.tile([128, CO, TQ], OUT_DT, tag="outc",
                                  name="outc", bufs=2)
                outc_ring[c] = outc
                tags = (("s2", 2), ("qkv", 1), ("oh", 1), ("yps", 1),
                        ("sps", 1)) if epilogue else (("oh", 1),)
                thunks = []
                for j in range(CO):
                    def mm(j=j):
                        tg, bf = tags[j % len(tags)]
                        oh = psum.tile([128, TQ], F32, tag=tg,
                                       name="oh", bufs=bf)
                        nc.tensor.matmul(
                            oh[:], wp_sb[:, j, :], yT[:],
                            start=True, stop=True)
                        if split_copy and j % 2 == 1:
                            nc.scalar.copy(outc[:, j, :], oh[:])
                        else:
                            nc.vector.tensor_copy(outc[:, j, :], oh[:])
                        if epilogue:
                            if j % 2 == 1:
                                nc.gpsimd.dma_start(
                                    outP_blk[:, j - 1 : j + 1, t0 : t0 + TQ],
                                    outc[:, j - 1 : j + 1, :])
                        elif j == 3:
                            nc.gpsimd.dma_start(
                                outP_blk[:, 0:4, t0 : t0 + TQ],
                                outc[:, 0:4, :])
                        elif j == CO - 1:
                            nc.gpsimd.dma_start(
                                outP_blk[:, 4:8, t0 : t0 + TQ],
                                outc[:, 4:8, :])
                    thunks.append(mm)
                return thunks

            def s2pair_f(qT, g):
                s2p = psum.tile([128, 2, TQ], F32, tag="s2", name="s2p",
                                bufs=2)
                for h in range(2):
                    si = 2 * g + h
                    nc.tensor.matmul(
                        s2p[:, h, :],
                        kT_sb[:, si * 128 : si * 128 + 128], qT[:],
                        start=True, stop=True)
                return s2p

            def exp_pair_f(g, s2p, n_s):
                diag = 2 * g >= n_s - 4
                if diag:
                    p2 = wpool.tile([128, 2, TQ], P_DT, tag="p2b",
                                    name="p2b", bufs=3)
                    nc.scalar.activation(
                        p2[:], s2p[:], mybir.ActivationFunctionType.Exp,
                        scale=EXP_SCALE)
                    for h in range(2):
                        si = 2 * g + h
                        nc.vector.tensor_mul(
                            p2[:, h, :], p2[:, h, :],
                            masks[:, si - (n_s - 4), :])
                else:
                    p2 = wpool.tile([128, 2, TQ], FP8, tag="p2f",
                                    name="p2f", bufs=4)
                    nc.scalar.activation(
                        p2[:], s2p[:], mybir.ActivationFunctionType.Exp,
                        scale=EXP_SCALE)
                return (p2, diag)

            # ---- prologue: QKV(0) dense ----------------------------------
            for th in qkv_thunks(0):
                th()

            # ---- main chunk loop -----------------------------------------
            hoisted = {}
            for c in range(nch):
                t0 = c * TQ
                n_s = 4 * (c + 1)
                n_p = n_s // 2

                filler = []
                if c + 1 < nch:
                    filler += qkv_thunks(c + 1)
                if c >= 1:
                    filler += cproj_thunks(c - 1, split_copy=(c - 1 <= 3))
                if c == 0:
                    xcs[2] = xc_fetch(2)
                if c + 3 < nch:
                    xcs[c + 3] = xc_fetch(c + 3)

                qT = qT_ring[c]
                v16 = v16_ring[c]
                y_ps = psum.tile([128, TQ], F32, tag="yps", name="y_ps",
                                 bufs=1)
                s_ps = psum.tile([128, TQ], F32, tag="sps", name="s_ps",
                                 bufs=1)

                pps = {}

                def ua_pair(g):
                    p2, diag = pps.pop(g)
                    if diag:
                        for h in range(2):
                            si = 2 * g + h
                            nc.tensor.matmul(
                                s_ps[:], ones_sq[:], p2[:, h, :],
                                start=(si == 0), stop=(si == n_s - 1))
                            nc.tensor.matmul(
                                y_ps[:], v16[:, si - (n_s - 4), :],
                                p2[:, h, :],
                                start=(si == 0), stop=(si == n_s - 1))
                    else:
                        nc.tensor.matmul(
                            s_ps[:], ones_dr[:], p2[:],
                            start=(g == 0), stop=False, perf_mode=DR)
                        nc.tensor.matmul(
                            y_ps[:], v_sb[:, 2 * g : 2 * g + 2, :], p2[:],
                            start=(g == 0), stop=False, perf_mode=DR)

                if c in hoisted:
                    pps[0] = hoisted.pop(c)
                else:
                    pps[0] = exp_pair_f(0, s2pair_f(qT, 0), n_s)
                for g in range(n_p):
                    if g + 1 < n_p:
                        s2_nxt = s2pair_f(qT, g + 1)
                    nf = len(filler)
                    if nf:
                        take = max(1, -(-nf // (n_p - g)))
                        for th in filler[:take]:
                            th()
                        del filler[:take]
                    if g + 1 < n_p:
                        pps[g + 1] = exp_pair_f(g + 1, s2_nxt, n_s)
                    elif c + 1 < nch:
                        # hoist the next chunk's first scores+exp here so
                        # its U/A never waits on a cold exp at the boundary
                        # (qT(c+1) was produced by this chunk's filler;
                        # kT block 0 is ancient)
                        s2h = s2pair_f(qT_ring[c + 1], 0)
                        hoisted[c + 1] = exp_pair_f(0, s2h, 4 * (c + 2))
                    ua_pair(g)

                for th in filler:
                    th()

                recip = wpool.tile([128, TQ], F32, tag="recip", name="recip",
                                   bufs=2)
                nc.vector.reciprocal_approx_fast(recip[:], s_ps[:])
                yT = wpool.tile([128, TQ], P_DT, tag="yT", name="yT", bufs=2)
                nc.vector.tensor_mul(yT[:], y_ps[:], recip[:])
                yT_ring[c] = yT

            # ---- epilogue: last chunk's c_proj, pipelined ----------------
            for th in cproj_thunks(nch - 1, split_copy=True, epilogue=True):
                th()

    nc.compile()
    return nc


def make_in_maps(x, w_attn, b_attn, w_proj, b_proj, t_len=T):
    """Shard + lay out the full inputs for the 8 cores."""
    x = np.asarray(x, dtype=np.float32).reshape(t_len, C)
    w_attn = np.asarray(w_attn, dtype=np.float32)
    b_attn = np.asarray(b_attn, dtype=np.float32)
    w_proj = np.asarray(w_proj, dtype=np.float32)

    scale = 1.0 / math.sqrt(D)
    fp8 = ml_dtypes.float8_e4m3
    bf16 = ml_dtypes.bfloat16
    xT = np.ascontiguousarray(x.T)
    xT8 = xT.astype(fp8)
    xT16 = np.ascontiguousarray(xT[:, :TQ]).astype(bf16)

    in_maps = []
    for h in range(N_CORES):
        sl = slice(h * D, (h + 1) * D)
        wq_s = (w_attn[sl, :] * (scale * SQ)).T
        wk_s = (w_attn[C + h * D : C + (h + 1) * D, :] * SK).T
        wv_s = (w_attn[2 * C + h * D : 2 * C + (h + 1) * D, :] * SV).T
        wp = np.ascontiguousarray((w_proj[:, sl] * (1.0 / SV)).T).astype(bf16)
        bqkv = np.stack(
            [
                b_attn[sl] * (scale * SQ),
                b_attn[C + h * D : C + (h + 1) * D] * SK,
                b_attn[2 * C + h * D : 2 * C + (h + 1) * D] * SV,
            ],
            axis=1,
        ).astype(np.float32)
        in_maps.append({
            "xT": xT8, "xT16": xT16,
            "wq": np.ascontiguousarray(wq_s).astype(fp8),
            "wk": np.ascontiguousarray(wk_s).astype(fp8),
            "wv": np.ascontiguousarray(wv_s).astype(fp8),
            "wq16": np.ascontiguousarray(wq_s).astype(bf16),
            "wk16": np.ascontiguousarray(wk_s).astype(bf16),
            "wv16": np.ascontiguousarray(wv_s).astype(bf16),
            "wp": wp,
            "bqkv": np.ascontiguousarray(bqkv),
        })
    return in_maps


_COMPILED = {}


def _get_compiled(t_len=T):
    if t_len not in _COMPILED:
        _COMPILED[t_len] = build(t_len)
    return _COMPILED[t_len]


def kernel(x, w_attn, b_attn, w_proj, b_proj, trace=False):
    nc = _get_compiled()
    in_maps = make_in_maps(x, w_attn, b_attn, w_proj, b_proj)
    res = bass_utils.run_bass_kernel_spmd(
        nc, in_maps, core_ids=list(range(N_CORES)), trace=trace
    )
    acc = res.results[0]["outP"].astype(np.float32)
    for h in range(1, N_CORES):
        acc += res.results[h]["outP"].astype(np.float32)
    out = acc.T + np.asarray(b_proj, dtype=np.float32)
    out = np.ascontiguousarray(out, dtype=np.float32).reshape(B, T, C)
    if trace:
        kernel.last_exec_time_ns = res.exec_time_ns
        kernel.last_results = res
    return out


# revision 25
# speedup vs baseline: 1.0883x; 1.0258x over previous
"""Causal self-attention (B=1, T=4096, C=1024, H=8) on 8 trn2 NeuronCores.

Tensor-parallel over heads: core h owns head h (D=128 = partition width).
Feature-major layout throughout: PE contraction dim always on SBUF
partitions.

Structure: 8 chunks of TQ=512 queries, software-pipelined attention
loop over s-tile PAIRS with fp8 DoubleRow matmuls where the
contraction dim allows pairing (2x PE columns/cycle):

  chunk c (queries t0=512c .. t0+511, s-tile pairs g = 0..2c+1):
    S(2g), S(2g+1) = kT-block.T @ qT        [PE bf16, 512 cols each]
    exp over the pair [128,1024] on ACT      (scale=1/(SQ*SK) folds the
                                              fp8 weight scaling out)
    clean pairs: p2 in fp8 -> U/A as DoubleRow fp8 matmuls (2 s-tiles
                 per instruction)
    diag pairs (last 2): p2 in bf16, DVE mask-mul, plain bf16 U/A
  emission per pair:  S(g+1) | filler MMs | exp(g+1) | U(g) A(g)

  QKV uses fp8 DoubleRow too (x and w_qkv in fp8, scaled by SQ/SK/SV
  to dodge fp8 subnormals; 1/(SQ*SK) folded into the exp scale,
  1/SV folded into w_proj on the host).

  Filler = QKV(c+1) matmuls + v(c+1) transposes + c_proj(c-1) matmuls,
  paced evenly across the pair loop so the PE never drains (keeps HAM
  at full clock).  DMA: inputs on the sync HWDGE ring; outputs (bf16
  partials, host sums in f32) on the gpsimd SWDGE ring.
"""

import math
import os
import sys

for _p in ("/opt/trn_rl_repo",):
    if _p not in sys.path:
        sys.path.insert(0, _p)

import numpy as np
import ml_dtypes

import concourse.bass as bass
import concourse.mybir as mybir
import concourse.tile as tile
from concourse import bacc
from concourse import bass_utils
from concourse.masks import make_identity

B, T, C, H = 1, 4096, 1024, 8
D = C // H          # 128, head dim == partition width
N_CORES = 8
TQ = 512            # query-chunk
NCH = T // TQ       # 8 chunks
CO = C // 128       # 8 contraction tiles of 128
F32 = mybir.dt.float32
BF16 = mybir.dt.bfloat16
FP8 = mybir.dt.float8e4
DR = mybir.MatmulPerfMode.DoubleRow

P_DT = BF16         # qT/kT storage
OUT_DT = BF16       # outP partial payload (host sums in f32)

# fp8 scaling: keep weight/act values out of e4m3 subnormals (<2^-6)
# and below the TRN e4m3 max of 240.
SQ = 64.0           # wq (incl 1/sqrt(D)) and bq
SK = 64.0           # wk, bk
SV = 32.0           # wv, bv; 1/SV folded into w_proj host-side
EXP_SCALE = 1.0 / (SQ * SK)


def build(t_len=T):
    """Emit the single-core SPMD program (same code on all 8 cores)."""
    n_ttiles = t_len // 128          # 32 s-tiles
    nch = t_len // TQ
    nc = bacc.Bacc(
        "TRN2", target_bir_lowering=False, debug=False, num_devices=N_CORES
    )

    xT_d = nc.dram_tensor("xT", [C, t_len], FP8, kind="ExternalInput")
    # chunk 0 runs QKV in bf16: its queries have few-term softmax
    # denominators, so fp8 projection error passes straight through
    xT16_d = nc.dram_tensor("xT16", [C, TQ], BF16, kind="ExternalInput")
    wq_d = nc.dram_tensor("wq", [C, D], FP8, kind="ExternalInput")
    wk_d = nc.dram_tensor("wk", [C, D], FP8, kind="ExternalInput")
    wv_d = nc.dram_tensor("wv", [C, D], FP8, kind="ExternalInput")
    wq16_d = nc.dram_tensor("wq16", [C, D], BF16, kind="ExternalInput")
    wk16_d = nc.dram_tensor("wk16", [C, D], BF16, kind="ExternalInput")
    wv16_d = nc.dram_tensor("wv16", [C, D], BF16, kind="ExternalInput")
    wp_d = nc.dram_tensor("wp", [D, C], BF16, kind="ExternalInput")
    bqkv_d = nc.dram_tensor("bqkv", [D, 3], F32, kind="ExternalInput")
    outP_d = nc.dram_tensor("outP", [C, t_len], OUT_DT, kind="ExternalOutput")

    with tile.TileContext(nc) as tc:
        with (
            tc.tile_pool(name="const", bufs=1) as cpool,
            tc.tile_pool(name="persist", bufs=1) as ppool,
            tc.tile_pool(name="work", bufs=2) as wpool,
            tc.tile_pool(name="psum", bufs=1, space="PSUM") as psum,
        ):
            # ---- weights / constants -------------------------------------
            wq_sb = cpool.tile([128, CO, D], FP8, name="wq_sb")
            wk_sb = cpool.tile([128, CO, D], FP8, name="wk_sb")
            wv_sb = cpool.tile([128, CO, D], FP8, name="wv_sb")
            wq16_sb = cpool.tile([128, CO, D], BF16, name="wq16_sb")
            wk16_sb = cpool.tile([128, CO, D], BF16, name="wk16_sb")
            wv16_sb = cpool.tile([128, CO, D], BF16, name="wv16_sb")
            wp_sb = cpool.tile([128, CO, D], BF16, name="wp_sb")
            bqkv_sb = cpool.tile([D, 3], F32, name="bqkv_sb")
            xc0_16 = cpool.tile([128, CO, TQ], BF16, name="xc0_16")
            # prologue inputs split across the two HWDGE rings (sync +
            # scalar) so the serialized per-DMA cost halves at the ramp
            nc.sync.dma_start(
                wq16_sb[:], wq16_d.ap().rearrange("(o p) m -> p o m", p=128)
            )
            nc.scalar.dma_start(bqkv_sb[:], bqkv_d.ap())
            nc.sync.dma_start(
                xc0_16[:, 0:4, :],
                xT16_d.ap().rearrange("(o p) t -> p o t", p=128)[:, 0:4, :])
            nc.sync.dma_start(
                xc0_16[:, 4:8, :],
                xT16_d.ap().rearrange("(o p) t -> p o t", p=128)[:, 4:8, :])
            for w_sb, w_d in ((wk16_sb, wk16_d), (wv16_sb, wv16_d)):
                nc.scalar.dma_start(
                    w_sb[:], w_d.ap().rearrange("(o p) m -> p o m", p=128)
                )

            xT_blk = xT_d.ap().rearrange("(o p) t -> p o t", p=128)
            outP_blk = outP_d.ap().rearrange("(o p) t -> p o t", p=128)

            # x chunk ring: [128, CO, TQ] per chunk, 3 deep
            def xc_fetch(c):
                xc = wpool.tile([128, CO, TQ], FP8, tag="xc", name="xc", bufs=4)
                t0 = c * TQ
                nc.sync.dma_start(xc[:, 0:4, :], xT_blk[:, 0:4, t0 : t0 + TQ])
                nc.sync.dma_start(xc[:, 4:8, :], xT_blk[:, 4:8, t0 : t0 + TQ])
                return xc

            for w_sb, w_d in ((wq_sb, wq_d), (wk_sb, wk_d), (wv_sb, wv_d)):
                nc.scalar.dma_start(
                    w_sb[:], w_d.ap().rearrange("(o p) m -> p o m", p=128)
                )
            xcs = {1: xc_fetch(1)}
            nc.scalar.dma_start(
                wp_sb[:], wp_d.ap().rearrange("d (o j) -> d o j", j=128)
            )

            masks = cpool.tile([128, 4, TQ], P_DT, name="masks")
            nc.vector.memset(masks[:], 1.0)
            for j in range(4):
                nc.gpsimd.affine_select(
                    out=masks[:, j, :], in_=masks[:, j, :],
                    compare_op=mybir.AluOpType.is_ge, fill=0.0,
                    base=-128 * j, pattern=[[1, TQ]], channel_multiplier=-1,
                )
            ones_sq = cpool.tile([128, 128], P_DT, name="ones_sq")
            nc.vector.memset(ones_sq[:], 1.0)
            ones_dr = cpool.tile([128, 2, 128], FP8, name="ones_dr")
            nc.vector.memset(ones_dr[:], 1.0)
            ident = cpool.tile([128, 128], P_DT, name="ident")
            make_identity(nc, ident[:])

            # HAM/ifetch warmup: dummy matmuls while input DMAs land
            warm_ps = psum.tile([128, 128], F32, tag="oh", name="warm_ps",
                                bufs=1)
            for wi in range(32):
                nc.tensor.matmul(warm_ps[:], ones_sq[:], ones_sq[:],
                                 start=True, stop=True)

            # ---- persistent activations ----------------------------------
            kT_sb = ppool.tile([128, t_len], P_DT, name="kT_sb")
            v_sb = ppool.tile([128, n_ttiles, D], FP8, name="v_sb")

            yT_ring = {}     # chunk -> yT tile [128, TQ]
            qT_ring = {}     # chunk -> qT tile [128, TQ]
            v16_ring = {}    # chunk -> bf16 v tiles [128, 4, 128] (diag)

            # ---------------- emission helpers ----------------------------
            def qkv_thunks(c):
                """Filler thunks computing q/k/v for chunk c from xc.
                Chunk 0 uses the bf16 path (precision: its queries have
                few-term denominators); later chunks use fp8 DoubleRow."""
                bf = c == 0
                xc = xc0_16 if bf else xcs[c]
                t0 = c * TQ
                thunks = []

                def proj(w_sb, kind):
                    ps = psum.tile([128, TQ], F32, tag="qkv",
                                   name=f"{kind}ps", bufs=1)
                    if bf:
                        for o in range(CO):
                            def mm(o=o, ps=ps, w_sb=w_sb):
                                nc.tensor.matmul(
                                    ps[:], w_sb[:, o, :], xc[:, o, :],
                                    start=(o == 0), stop=(o == CO - 1),
                                )
                            thunks.append(mm)
                    else:
                        for o2 in range(4):
                            def mm(o2=o2, ps=ps, w_sb=w_sb):
                                nc.tensor.matmul(
                                    ps[:], w_sb[:, 2 * o2 : 2 * o2 + 2, :],
                                    xc[:, 2 * o2 : 2 * o2 + 2, :],
                                    start=(o2 == 0), stop=(o2 == 3),
                                    perf_mode=DR,
                                )
                            thunks.append(mm)

                    def finish(ps=ps, kind=kind):
                        if kind == "q":
                            qT = wpool.tile([128, TQ], P_DT, tag="qT",
                                            name="qT", bufs=2)
                            nc.vector.tensor_add(
                                qT[:], ps[:],
                                bqkv_sb[:, 0:1].to_broadcast([D, TQ]))
                            qT_ring[c] = qT
                        elif kind == "k":
                            nc.vector.tensor_add(
                                kT_sb[:, t0 : t0 + TQ], ps[:],
                                bqkv_sb[:, 1:2].to_broadcast([D, TQ]))
                        else:
                            vT = wpool.tile([128, TQ], P_DT, tag="vT",
                                            name="vT", bufs=2)
                            nc.vector.tensor_add(
                                vT[:], ps[:],
                                bqkv_sb[:, 2:3].to_broadcast([D, TQ]))
                            # transpose to token-major [s, d] tiles
                            vt_ps = psum.tile([128, 4, 128], P_DT, tag="oh",
                                              name="vt_ps", bufs=1)
                            for tt in range(4):
                                nc.tensor.transpose(
                                    vt_ps[:, tt, :],
                                    vT[:, tt * 128 : (tt + 1) * 128],
                                    ident[:])
                            # fp8 copy for DoubleRow A/U; bf16 copy for
                            # the diagonal (masked) pairs of chunk c
                            nc.vector.tensor_copy(
                                v_sb[:, 4 * c : 4 * c + 4, :], vt_ps[:])
                            if c == 0:
                                v16 = wpool.tile([128, 4, 128], P_DT,
                                                 tag="v16", name="v16",
                                                 bufs=1)
                                nc.vector.tensor_copy(v16[:], vt_ps[:])
                                v16_ring[c] = v16
                    # attach the finish to the last MM thunk
                    last = thunks.pop()
                    def last_plus(last=last, finish=finish):
                        last()
                        finish()
                    thunks.append(last_plus)

                if bf:
                    proj(wq16_sb, "q")
                    proj(wk16_sb, "k")
                    proj(wv16_sb, "v")
                else:
                    proj(wq_sb, "q")
                    proj(wk_sb, "k")
                    proj(wv_sb, "v")
                return thunks

            outc_ring = {}

            def cproj_thunks(c, split_copy=False, epilogue=False):
                """Filler thunks computing the local c_proj partial of
                chunk c (host sums partials over cores).  The epilogue
                variant round-robins psum banks across the now-idle tags
                and quarters the output DMA to shrink the receipt tail."""
                t0 = c * TQ
                yT = yT_ring[c]
                outc = wpool.tile([128, CO, TQ], OUT_DT, tag="outc",
                                  name="outc", bufs=2)
                outc_ring[c] = outc
                tags = (("s2", 2), ("qkv", 1), ("oh", 1), ("yps", 1),
                        ("sps", 1)) if epilogue else (("oh", 1),)
                thunks = []
                for j in range(CO):
                    def mm(j=j):
                        tg, bf = tags[j % len(tags)]
                        oh = psum.tile([128, TQ], F32, tag=tg,
                                       name="oh", bufs=bf)
                        nc.tensor.matmul(
                            oh[:], wp_sb[:, j, :], yT[:],
                            start=True, stop=True)
                        if split_copy and j % 2 == 1:
                            nc.scalar.copy(outc[:, j, :], oh[:])
                        else:
                            nc.vector.tensor_copy(outc[:, j, :], oh[:])
                        if epilogue:
                            if j % 2 == 1:
                                nc.gpsimd.dma_start(
                                    outP_blk[:, j - 1 : j + 1, t0 : t0 + TQ],
                                    outc[:, j - 1 : j + 1, :])
                        elif j == 3:
                            nc.gpsimd.dma_start(
                                outP_blk[:, 0:4, t0 : t0 + TQ],
                                outc[:, 0:4, :])
                        elif j == CO - 1:
                            nc.gpsimd.dma_start(
                                outP_blk[:, 4:8, t0 : t0 + TQ],
                                outc[:, 4:8, :])
                    thunks.append(mm)
                return thunks

            def s2pair_f(qT, g):
                s2p = psum.tile([128, 2, TQ], F32, tag="s2", name="s2p",
                                bufs=2)
                for h in range(2):
                    si = 2 * g + h
                    nc.tensor.matmul(
                        s2p[:, h, :],
                        kT_sb[:, si * 128 : si * 128 + 128], qT[:],
                        start=True, stop=True)
                return s2p

            def exp_pair_f(g, s2p, n_s):
                # bf16 U/A only for chunk 0 (few-term denominators);
                # later chunks run even their diagonal (masked) pairs
                # through the fp8 DoubleRow path — verified numerically.
                diag = 2 * g >= n_s - 4
                bf = n_s <= 4
                if bf:
                    p2 = wpool.tile([128, 2, TQ], P_DT, tag="p2b",
                                    name="p2b", bufs=3)
                else:
                    p2 = wpool.tile([128, 2, TQ], FP8, tag="p2f",
                                    name="p2f", bufs=4)
                nc.scalar.activation(
                    p2[:], s2p[:], mybir.ActivationFunctionType.Exp,
                    scale=EXP_SCALE)
                if diag:
                    for h in range(2):
                        si = 2 * g + h
                        nc.vector.tensor_mul(
                            p2[:, h, :], p2[:, h, :],
                            masks[:, si - (n_s - 4), :])
                return (p2, bf)

            # ---- prologue: QKV(0) dense ----------------------------------
            for th in qkv_thunks(0):
                th()

            # ---- main chunk loop -----------------------------------------
            hoisted = {}
            for c in range(nch):
                t0 = c * TQ
                n_s = 4 * (c + 1)
                n_p = n_s // 2

                filler = []
                if c + 1 < nch:
                    filler += qkv_thunks(c + 1)
                if c >= 1:
                    filler += cproj_thunks(c - 1, split_copy=(c - 1 <= 3))
                if c == 0:
                    xcs[2] = xc_fetch(2)
                if c + 3 < nch:
                    xcs[c + 3] = xc_fetch(c + 3)

                qT = qT_ring[c]
                v16 = v16_ring.get(c)
                y_ps = psum.tile([128, TQ], F32, tag="yps", name="y_ps",
                                 bufs=1)
                s_ps = psum.tile([128, TQ], F32, tag="sps", name="s_ps",
                                 bufs=1)

                pps = {}

                def ua_pair(g):
                    p2, bf = pps.pop(g)
                    if bf:
                        for h in range(2):
                            si = 2 * g + h
                            nc.tensor.matmul(
                                s_ps[:], ones_sq[:], p2[:, h, :],
                                start=(si == 0), stop=(si == n_s - 1))
                            nc.tensor.matmul(
                                y_ps[:], v16[:, si - (n_s - 4), :],
                                p2[:, h, :],
                                start=(si == 0), stop=(si == n_s - 1))
                    else:
                        nc.tensor.matmul(
                            s_ps[:], ones_dr[:], p2[:],
                            start=(g == 0), stop=(g == n_p - 1),
                            perf_mode=DR)
                        nc.tensor.matmul(
                            y_ps[:], v_sb[:, 2 * g : 2 * g + 2, :], p2[:],
                            start=(g == 0), stop=(g == n_p - 1),
                            perf_mode=DR)

                if c in hoisted:
                    pps[0] = hoisted.pop(c)
                else:
                    pps[0] = exp_pair_f(0, s2pair_f(qT, 0), n_s)
                for g in range(n_p):
                    if g + 1 < n_p:
                        s2_nxt = s2pair_f(qT, g + 1)
                    nf = len(filler)
                    if nf:
                        take = max(1, -(-nf // (n_p - g)))
                        for th in filler[:take]:
                            th()
                        del filler[:take]
                    if g + 1 < n_p:
                        pps[g + 1] = exp_pair_f(g + 1, s2_nxt, n_s)
                    elif c + 1 < nch:
                        # hoist the next chunk's first scores+exp here so
                        # its U/A never waits on a cold exp at the boundary
                        # (qT(c+1) was produced by this chunk's filler;
                        # kT block 0 is ancient)
                        s2h = s2pair_f(qT_ring[c + 1], 0)
                        hoisted[c + 1] = exp_pair_f(0, s2h, 4 * (c + 2))
                    ua_pair(g)

                for th in filler:
                    th()

                recip = wpool.tile([128, TQ], F32, tag="recip", name="recip",
                                   bufs=2)
                nc.vector.reciprocal_approx_fast(recip[:], s_ps[:])
                yT = wpool.tile([128, TQ], P_DT, tag="yT", name="yT", bufs=2)
                nc.vector.tensor_mul(yT[:], y_ps[:], recip[:])
                yT_ring[c] = yT

            # ---- epilogue: last chunk's c_proj, pipelined ----------------
            for th in cproj_thunks(nch - 1, split_copy=True, epilogue=True):
                th()

    nc.compile()
    return nc


def make_in_maps(x, w_attn, b_attn, w_proj, b_proj, t_len=T):
    """Shard + lay out the full inputs for the 8 cores."""
    x = np.asarray(x, dtype=np.float32).reshape(t_len, C)
    w_attn = np.asarray(w_attn, dtype=np.float32)
    b_attn = np.asarray(b_attn, dtype=np.float32)
    w_proj = np.asarray(w_proj, dtype=np.float32)

    scale = 1.0 / math.sqrt(D)
    fp8 = ml_dtypes.float8_e4m3
    bf16 = ml_dtypes.bfloat16
    xT = np.ascontiguousarray(x.T)
    xT8 = xT.astype(fp8)
    xT16 = np.ascontiguousarray(xT[:, :TQ]).astype(bf16)

    in_maps = []
    for h in range(N_CORES):
        sl = slice(h * D, (h + 1) * D)
        wq_s = (w_attn[sl, :] * (scale * SQ)).T
        wk_s = (w_attn[C + h * D : C + (h + 1) * D, :] * SK).T
        wv_s = (w_attn[2 * C + h * D : 2 * C + (h + 1) * D, :] * SV).T
        wp = np.ascontiguousarray((w_proj[:, sl] * (1.0 / SV)).T).astype(bf16)
        bqkv = np.stack(
            [
                b_attn[sl] * (scale * SQ),
                b_attn[C + h * D : C + (h + 1) * D] * SK,
                b_attn[2 * C + h * D : 2 * C + (h + 1) * D] * SV,
            ],
            axis=1,
        ).astype(np.float32)
        in_maps.append({
            "xT": xT8, "xT16": xT16,
            "wq": np.ascontiguousarray(wq_s).astype(fp8),
            "wk": np.ascontiguousarray(wk_s).astype(fp8),
            "wv": np.ascontiguousarray(wv_s).astype(fp8),
            "wq16": np.ascontiguousarray(wq_s).astype(bf16),
            "wk16": np.ascontiguousarray(wk_s).astype(bf16),
            "wv16": np.ascontiguousarray(wv_s).astype(bf16),
            "wp": wp,
            "bqkv": np.ascontiguousarray(bqkv),
        })
    return in_maps


_COMPILED = {}


def _get_compiled(t_len=T):
    if t_len not in _COMPILED:
        _COMPILED[t_len] = build(t_len)
    return _COMPILED[t_len]


def kernel(x, w_attn, b_attn, w_proj, b_proj, trace=False):
    nc = _get_compiled()
    in_maps = make_in_maps(x, w_attn, b_attn, w_proj, b_proj)
    res = bass_utils.run_bass_kernel_spmd(
        nc, in_maps, core_ids=list(range(N_CORES)), trace=trace
    )
    acc = res.results[0]["outP"].astype(np.float32)
    for h in range(1, N_CORES):
        acc += res.results[h]["outP"].astype(np.float32)
    out = acc.T + np.asarray(b_proj, dtype=np.float32)
    out = np.ascontiguousarray(out, dtype=np.float32).reshape(B, T, C)
    if trace:
        kernel.last_exec_time_ns = res.exec_time_ns
        kernel.last_results = res
    return out


# revision 28
# speedup vs baseline: 1.0970x; 1.0080x over previous
"""Causal self-attention (B=1, T=4096, C=1024, H=8) on 8 trn2 NeuronCores.

Tensor-parallel over heads: core h owns head h (D=128 = partition width).
Feature-major layout throughout: PE contraction dim always on SBUF
partitions.

Structure: 8 chunks of TQ=512 queries, software-pipelined attention
loop over s-tile PAIRS with fp8 DoubleRow matmuls where the
contraction dim allows pairing (2x PE columns/cycle):

  chunk c (queries t0=512c .. t0+511, s-tile pairs g = 0..2c+1):
    S(2g), S(2g+1) = kT-block.T @ qT        [PE bf16, 512 cols each]
    exp over the pair [128,1024] on ACT      (scale=1/(SQ*SK) folds the
                                              fp8 weight scaling out)
    clean pairs: p2 in fp8 -> U/A as DoubleRow fp8 matmuls (2 s-tiles
                 per instruction)
    diag pairs (last 2): p2 in bf16, DVE mask-mul, plain bf16 U/A
  emission per pair:  S(g+1) | filler MMs | exp(g+1) | U(g) A(g)

  QKV uses fp8 DoubleRow too (x and w_qkv in fp8, scaled by SQ/SK/SV
  to dodge fp8 subnormals; 1/(SQ*SK) folded into the exp scale,
  1/SV folded into w_proj on the host).

  Filler = QKV(c+1) matmuls + v(c+1) transposes + c_proj(c-1) matmuls,
  paced evenly across the pair loop so the PE never drains (keeps HAM
  at full clock).  DMA: inputs on the sync HWDGE ring; outputs (bf16
  partials, host sums in f32) on the gpsimd SWDGE ring.
"""

import math
import os
import sys

for _p in ("/opt/trn_rl_repo",):
    if _p not in sys.path:
        sys.path.insert(0, _p)

import numpy as np
import ml_dtypes

import concourse.bass as bass
import concourse.mybir as mybir
import concourse.tile as tile
from concourse import bacc
from concourse import bass_utils
from concourse.masks import make_identity

B, T, C, H = 1, 4096, 1024, 8
D = C // H          # 128, head dim == partition width
N_CORES = 8
TQ = 512            # query-chunk
NCH = T // TQ       # 8 chunks
CO = C // 128       # 8 contraction tiles of 128
F32 = mybir.dt.float32
BF16 = mybir.dt.bfloat16
FP8 = mybir.dt.float8e4
DR = mybir.MatmulPerfMode.DoubleRow

P_DT = BF16         # qT/kT storage
OUT_DT = BF16       # outP partial payload (host sums in f32)

# fp8 scaling: keep weight/act values out of e4m3 subnormals (<2^-6)
# and below the TRN e4m3 max of 240.
SQ = 64.0           # wq (incl 1/sqrt(D)) and bq
SK = 64.0           # wk, bk
SV = 32.0           # wv, bv; 1/SV folded into w_proj host-side
EXP_SCALE = 1.0 / (SQ * SK)


def build(t_len=T):
    """Emit the single-core SPMD program (same code on all 8 cores)."""
    n_ttiles = t_len // 128          # 32 s-tiles
    nch = t_len // TQ
    nc = bacc.Bacc(
        "TRN2", target_bir_lowering=False, debug=False, num_devices=N_CORES
    )

    xT_d = nc.dram_tensor("xT", [C, t_len], FP8, kind="ExternalInput")
    # chunk 0 runs QKV in bf16: its queries have few-term softmax
    # denominators, so fp8 projection error passes straight through
    xT16_d = nc.dram_tensor("xT16", [C, TQ], BF16, kind="ExternalInput")
    wq_d = nc.dram_tensor("wq", [C, D], FP8, kind="ExternalInput")
    wk_d = nc.dram_tensor("wk", [C, D], FP8, kind="ExternalInput")
    wv_d = nc.dram_tensor("wv", [C, D], FP8, kind="ExternalInput")
    wq16_d = nc.dram_tensor("wq16", [C, D], BF16, kind="ExternalInput")
    wk16_d = nc.dram_tensor("wk16", [C, D], BF16, kind="ExternalInput")
    wv16_d = nc.dram_tensor("wv16", [C, D], BF16, kind="ExternalInput")
    wp_d = nc.dram_tensor("wp", [D, C], BF16, kind="ExternalInput")
    bqkv_d = nc.dram_tensor("bqkv", [D, 3], F32, kind="ExternalInput")
    outP_d = nc.dram_tensor("outP", [C, t_len], OUT_DT, kind="ExternalOutput")

    with tile.TileContext(nc) as tc:
        with (
            tc.tile_pool(name="const", bufs=1) as cpool,
            tc.tile_pool(name="persist", bufs=1) as ppool,
            tc.tile_pool(name="work", bufs=2) as wpool,
            tc.tile_pool(name="psum", bufs=1, space="PSUM") as psum,
        ):
            # ---- weights / constants -------------------------------------
            wq_sb = cpool.tile([128, CO, D], FP8, name="wq_sb")
            wk_sb = cpool.tile([128, CO, D], FP8, name="wk_sb")
            wv_sb = cpool.tile([128, CO, D], FP8, name="wv_sb")
            wq16_sb = cpool.tile([128, CO, D], BF16, name="wq16_sb")
            wk16_sb = cpool.tile([128, CO, D], BF16, name="wk16_sb")
            wv16_sb = cpool.tile([128, CO, D], BF16, name="wv16_sb")
            wp_sb = cpool.tile([128, CO, D], BF16, name="wp_sb")
            bqkv_sb = cpool.tile([D, 3], F32, name="bqkv_sb")
            xc0_16 = cpool.tile([128, CO, TQ], BF16, name="xc0_16")
            # prologue inputs split across the two HWDGE rings (sync +
            # scalar) so the serialized per-DMA cost halves at the ramp
            nc.sync.dma_start(
                wq16_sb[:], wq16_d.ap().rearrange("(o p) m -> p o m", p=128)
            )
            nc.scalar.dma_start(bqkv_sb[:], bqkv_d.ap())
            nc.sync.dma_start(
                xc0_16[:, 0:4, :],
                xT16_d.ap().rearrange("(o p) t -> p o t", p=128)[:, 0:4, :])
            nc.scalar.dma_start(
                xc0_16[:, 4:8, :],
                xT16_d.ap().rearrange("(o p) t -> p o t", p=128)[:, 4:8, :])
            for w_sb, w_d in ((wk16_sb, wk16_d), (wv16_sb, wv16_d)):
                nc.scalar.dma_start(
                    w_sb[:], w_d.ap().rearrange("(o p) m -> p o m", p=128)
                )

            xT_blk = xT_d.ap().rearrange("(o p) t -> p o t", p=128)
            outP_blk = outP_d.ap().rearrange("(o p) t -> p o t", p=128)

            # x chunk ring: [128, CO, TQ] per chunk, 3 deep
            def xc_fetch(c):
                xc = wpool.tile([128, CO, TQ], FP8, tag="xc", name="xc", bufs=4)
                t0 = c * TQ
                nc.sync.dma_start(xc[:, 0:4, :], xT_blk[:, 0:4, t0 : t0 + TQ])
                nc.sync.dma_start(xc[:, 4:8, :], xT_blk[:, 4:8, t0 : t0 + TQ])
                return xc

            for w_sb, w_d in ((wq_sb, wq_d), (wk_sb, wk_d), (wv_sb, wv_d)):
                nc.scalar.dma_start(
                    w_sb[:], w_d.ap().rearrange("(o p) m -> p o m", p=128)
                )
            xcs = {1: xc_fetch(1)}
            nc.scalar.dma_start(
                wp_sb[:], wp_d.ap().rearrange("d (o j) -> d o j", j=128)
            )

            masks = cpool.tile([128, 4, TQ], P_DT, name="masks")
            nc.vector.memset(masks[:], 1.0)
            for j in range(4):
                nc.gpsimd.affine_select(
                    out=masks[:, j, :], in_=masks[:, j, :],
                    compare_op=mybir.AluOpType.is_ge, fill=0.0,
                    base=-128 * j, pattern=[[1, TQ]], channel_multiplier=-1,
                )
            ones_sq = cpool.tile([128, 128], P_DT, name="ones_sq")
            nc.vector.memset(ones_sq[:], 1.0)
            ones_dr = cpool.tile([128, 2, 128], FP8, name="ones_dr")
            nc.vector.memset(ones_dr[:], 1.0)
            ident = cpool.tile([128, 128], P_DT, name="ident")
            make_identity(nc, ident[:])

            # HAM/ifetch warmup: dummy matmuls while input DMAs land
            warm_ps = psum.tile([128, 128], F32, tag="oh", name="warm_ps",
                                bufs=1)
            for wi in range(26):
                nc.tensor.matmul(warm_ps[:], ones_sq[:], ones_sq[:],
                                 start=True, stop=True)

            # ---- persistent activations ----------------------------------
            kT_sb = ppool.tile([128, t_len], P_DT, name="kT_sb")
            v_sb = ppool.tile([128, n_ttiles, D], FP8, name="v_sb")

            yT_ring = {}     # chunk -> yT tile [128, TQ]
            qT_ring = {}     # chunk -> qT tile [128, TQ]
            v16_ring = {}    # chunk -> bf16 v tiles [128, 4, 128] (diag)

            # ---------------- emission helpers ----------------------------
            def qkv_thunks(c):
                """Filler thunks computing q/k/v for chunk c from xc.
                Chunk 0 uses the bf16 path (precision: its queries have
                few-term denominators); later chunks use fp8 DoubleRow."""
                bf = c == 0
                xc = xc0_16 if bf else xcs[c]
                t0 = c * TQ
                thunks = []

                def proj(w_sb, kind):
                    ps = psum.tile([128, TQ], F32, tag="qkv",
                                   name=f"{kind}ps", bufs=1)
                    if bf:
                        for o in range(CO):
                            def mm(o=o, ps=ps, w_sb=w_sb):
                                nc.tensor.matmul(
                                    ps[:], w_sb[:, o, :], xc[:, o, :],
                                    start=(o == 0), stop=(o == CO - 1),
                                )
                            thunks.append(mm)
                    else:
                        for o2 in range(4):
                            def mm(o2=o2, ps=ps, w_sb=w_sb):
                                nc.tensor.matmul(
                                    ps[:], w_sb[:, 2 * o2 : 2 * o2 + 2, :],
                                    xc[:, 2 * o2 : 2 * o2 + 2, :],
                                    start=(o2 == 0), stop=(o2 == 3),
                                    perf_mode=DR,
                                )
                            thunks.append(mm)

                    def finish(ps=ps, kind=kind):
                        if kind == "q":
                            qT = wpool.tile([128, TQ], P_DT, tag="qT",
                                            name="qT", bufs=2)
                            nc.vector.tensor_add(
                                qT[:], ps[:],
                                bqkv_sb[:, 0:1].to_broadcast([D, TQ]))
                            qT_ring[c] = qT
                        elif kind == "k":
                            nc.vector.tensor_add(
                                kT_sb[:, t0 : t0 + TQ], ps[:],
                                bqkv_sb[:, 1:2].to_broadcast([D, TQ]))
                        else:
                            vT = wpool.tile([128, TQ], P_DT, tag="vT",
                                            name="vT", bufs=2)
                            nc.vector.tensor_add(
                                vT[:], ps[:],
                                bqkv_sb[:, 2:3].to_broadcast([D, TQ]))
                            # transpose to token-major [s, d] tiles
                            vt_ps = psum.tile([128, 4, 128], P_DT, tag="oh",
                                              name="vt_ps", bufs=1)
                            for tt in range(4):
                                nc.tensor.transpose(
                                    vt_ps[:, tt, :],
                                    vT[:, tt * 128 : (tt + 1) * 128],
                                    ident[:])
                            # fp8 copy for DoubleRow A/U; bf16 copy for
                            # the diagonal (masked) pairs of chunk c
                            nc.vector.tensor_copy(
                                v_sb[:, 4 * c : 4 * c + 4, :], vt_ps[:])
                            if c == 0:
                                v16 = wpool.tile([128, 4, 128], P_DT,
                                                 tag="v16", name="v16",
                                                 bufs=1)
                                nc.vector.tensor_copy(v16[:], vt_ps[:])
                                v16_ring[c] = v16
                    # attach the finish to the last MM thunk
                    last = thunks.pop()
                    def last_plus(last=last, finish=finish):
                        last()
                        finish()
                    thunks.append(last_plus)

                if bf:
                    proj(wq16_sb, "q")
                    proj(wk16_sb, "k")
                    proj(wv16_sb, "v")
                else:
                    proj(wq_sb, "q")
                    proj(wk_sb, "k")
                    proj(wv_sb, "v")
                return thunks

            outc_ring = {}

            def cproj_thunks(c, split_copy=False, epilogue=False):
                """Filler thunks computing the local c_proj partial of
                chunk c (host sums partials over cores).  The epilogue
                variant round-robins psum banks across the now-idle tags
                and quarters the output DMA to shrink the receipt tail."""
                t0 = c * TQ
                yT = yT_ring[c]
                outc = wpool.tile([128, CO, TQ], OUT_DT, tag="outc",
                                  name="outc", bufs=2)
                outc_ring[c] = outc
                tags = (("s2", 2), ("qkv", 1), ("oh", 1), ("yps", 1),
                        ("sps", 1)) if epilogue else (("oh", 1),)
                thunks = []
                for j in range(CO):
                    def mm(j=j):
                        tg, bf = tags[j % len(tags)]
                        oh = psum.tile([128, TQ], F32, tag=tg,
                                       name="oh", bufs=bf)
                        nc.tensor.matmul(
                            oh[:], wp_sb[:, j, :], yT[:],
                            start=True, stop=True)
                        if split_copy and j % 2 == 1:
                            nc.scalar.copy(outc[:, j, :], oh[:])
                        else:
                            nc.vector.tensor_copy(outc[:, j, :], oh[:])
                        if epilogue:
                            # early blocks in pairs, last blocks singly so
                            # the final DMA (and its completion receipt)
                            # covers only 0.125 MB
                            if j in (1, 3):
                                nc.gpsimd.dma_start(
                                    outP_blk[:, j - 1 : j + 1, t0 : t0 + TQ],
                                    outc[:, j - 1 : j + 1, :])
                            elif j >= 4:
                                nc.gpsimd.dma_start(
                                    outP_blk[:, j : j + 1, t0 : t0 + TQ],
                                    outc[:, j : j + 1, :])
                        elif j == 3:
                            nc.gpsimd.dma_start(
                                outP_blk[:, 0:4, t0 : t0 + TQ],
                                outc[:, 0:4, :])
                        elif j == CO - 1:
                            nc.gpsimd.dma_start(
                                outP_blk[:, 4:8, t0 : t0 + TQ],
                                outc[:, 4:8, :])
                    thunks.append(mm)
                return thunks

            def s2pair_f(qT, g):
                s2p = psum.tile([128, 2, TQ], F32, tag="s2", name="s2p",
                                bufs=2)
                for h in range(2):
                    si = 2 * g + h
                    nc.tensor.matmul(
                        s2p[:, h, :],
                        kT_sb[:, si * 128 : si * 128 + 128], qT[:],
                        start=True, stop=True)
                return s2p

            def exp_pair_f(g, s2p, n_s):
                # bf16 U/A only for chunk 0 (few-term denominators);
                # later chunks run even their diagonal (masked) pairs
                # through the fp8 DoubleRow path — verified numerically.
                diag = 2 * g >= n_s - 4
                bf = n_s <= 4
                if bf:
                    p2 = wpool.tile([128, 2, TQ], P_DT, tag="p2b",
                                    name="p2b", bufs=3)
                else:
                    p2 = wpool.tile([128, 2, TQ], FP8, tag="p2f",
                                    name="p2f", bufs=4)
                nc.scalar.activation(
                    p2[:], s2p[:], mybir.ActivationFunctionType.Exp,
                    scale=EXP_SCALE)
                if diag:
                    for h in range(2):
                        si = 2 * g + h
                        nc.vector.tensor_mul(
                            p2[:, h, :], p2[:, h, :],
                            masks[:, si - (n_s - 4), :])
                return (p2, bf)

            # ---- prologue: QKV(0) dense ----------------------------------
            for th in qkv_thunks(0):
                th()

            # ---- main chunk loop -----------------------------------------
            hoisted = {}
            for c in range(nch):
                t0 = c * TQ
                n_s = 4 * (c + 1)
                n_p = n_s // 2

                filler = []
                if c + 1 < nch:
                    filler += qkv_thunks(c + 1)
                if c >= 1:
                    filler += cproj_thunks(c - 1, split_copy=(c - 1 <= 3))
                if c == 0:
                    xcs[2] = xc_fetch(2)
                if c + 3 < nch:
                    xcs[c + 3] = xc_fetch(c + 3)

                qT = qT_ring[c]
                v16 = v16_ring.get(c)
                y_ps = psum.tile([128, TQ], F32, tag="yps", name="y_ps",
                                 bufs=1)
                s_ps = psum.tile([128, TQ], F32, tag="sps", name="s_ps",
                                 bufs=1)

                pps = {}

                def ua_pair(g):
                    p2, bf = pps.pop(g)
                    if bf:
                        for h in range(2):
                            si = 2 * g + h
                            nc.tensor.matmul(
                                s_ps[:], ones_sq[:], p2[:, h, :],
                                start=(si == 0), stop=(si == n_s - 1))
                            nc.tensor.matmul(
                                y_ps[:], v16[:, si - (n_s - 4), :],
                                p2[:, h, :],
                                start=(si == 0), stop=(si == n_s - 1))
                    else:
                        nc.tensor.matmul(
                            s_ps[:], ones_dr[:], p2[:],
                            start=(g == 0), stop=(g == n_p - 1),
                            perf_mode=DR)
                        nc.tensor.matmul(
                            y_ps[:], v_sb[:, 2 * g : 2 * g + 2, :], p2[:],
                            start=(g == 0), stop=(g == n_p - 1),
                            perf_mode=DR)

                if c in hoisted:
                    pps[0] = hoisted.pop(c)
                else:
                    pps[0] = exp_pair_f(0, s2pair_f(qT, 0), n_s)
                for g in range(n_p):
                    if g + 1 < n_p:
                        s2_nxt = s2pair_f(qT, g + 1)
                    nf = len(filler)
                    if nf:
                        take = max(1, -(-nf // (n_p - g)))
                        for th in filler[:take]:
                            th()
                        del filler[:take]
                    if g + 1 < n_p:
                        pps[g + 1] = exp_pair_f(g + 1, s2_nxt, n_s)
                    elif c + 1 < nch:
                        # hoist the next chunk's first scores+exp here so
                        # its U/A never waits on a cold exp at the boundary
                        # (qT(c+1) was produced by this chunk's filler;
                        # kT block 0 is ancient)
                        s2h = s2pair_f(qT_ring[c + 1], 0)
                        hoisted[c + 1] = exp_pair_f(0, s2h, 4 * (c + 2))
                    ua_pair(g)

                for th in filler:
                    th()

                recip = wpool.tile([128, TQ], F32, tag="recip", name="recip",
                                   bufs=2)
                nc.vector.reciprocal_approx_fast(recip[:], s_ps[:])
                yT = wpool.tile([128, TQ], P_DT, tag="yT", name="yT", bufs=2)
                nc.vector.tensor_mul(yT[:], y_ps[:], recip[:])
                yT_ring[c] = yT

            # ---- epilogue: last chunk's c_proj, pipelined ----------------
            for th in cproj_thunks(nch - 1, split_copy=True, epilogue=True):
                th()

    nc.compile()
    return nc


def make_in_maps(x, w_attn, b_attn, w_proj, b_proj, t_len=T):
    """Shard + lay out the full inputs for the 8 cores."""
    x = np.asarray(x, dtype=np.float32).reshape(t_len, C)
    w_attn = np.asarray(w_attn, dtype=np.float32)
    b_attn = np.asarray(b_attn, dtype=np.float32)
    w_proj = np.asarray(w_proj, dtype=np.float32)

    scale = 1.0 / math.sqrt(D)
    fp8 = ml_dtypes.float8_e4m3
    bf16 = ml_dtypes.bfloat16
    xT = np.ascontiguousarray(x.T)
    xT8 = xT.astype(fp8)
    xT16 = np.ascontiguousarray(xT[:, :TQ]).astype(bf16)

    in_maps = []
    for h in range(N_CORES):
        sl = slice(h * D, (h + 1) * D)
        wq_s = (w_attn[sl, :] * (scale * SQ)).T
        wk_s = (w_attn[C + h * D : C + (h + 1) * D, :] * SK).T
        wv_s = (w_attn[2 * C + h * D : 2 * C + (h + 1) * D, :] * SV).T
        wp = np.ascontiguousarray((w_proj[:, sl] * (1.0 / SV)).T).astype(bf16)
        bqkv = np.stack(
            [
                b_attn[sl] * (scale * SQ),
                b_attn[C + h * D : C + (h + 1) * D] * SK,
                b_attn[2 * C + h * D : 2 * C + (h + 1) * D] * SV,
            ],
            axis=1,
        ).astype(np.float32)
        in_maps.append({
            "xT": xT8, "xT16": xT16,
            "wq": np.ascontiguousarray(wq_s).astype(fp8),
            "wk": np.ascontiguousarray(wk_s).astype(fp8),
            "wv": np.ascontiguousarray(wv_s).astype(fp8),
            "wq16": np.ascontiguousarray(wq_s).astype(bf16),
            "wk16": np.ascontiguousarray(wk_s).astype(bf16),
            "wv16": np.ascontiguousarray(wv_s).astype(bf16),
            "wp": wp,
            "bqkv": np.ascontiguousarray(bqkv),
        })
    return in_maps


_COMPILED = {}


def _get_compiled(t_len=T):
    if t_len not in _COMPILED:
        _COMPILED[t_len] = build(t_len)
    return _COMPILED[t_len]


def kernel(x, w_attn, b_attn, w_proj, b_proj, trace=False):
    nc = _get_compiled()
    in_maps = make_in_maps(x, w_attn, b_attn, w_proj, b_proj)
    res = bass_utils.run_bass_kernel_spmd(
        nc, in_maps, core_ids=list(range(N_CORES)), trace=trace
    )
    acc = res.results[0]["outP"].astype(np.float32)
    for h in range(1, N_CORES):
        acc += res.results[h]["outP"].astype(np.float32)
    out = acc.T + np.asarray(b_proj, dtype=np.float32)
    out = np.ascontiguousarray(out, dtype=np.float32).reshape(B, T, C)
    if trace:
        kernel.last_exec_time_ns = res.exec_time_ns
        kernel.last_results = res
    return out
